# revision 23
# baseline (speedup 1.0000x reference)
"""Trainium2 Bass kernel for nn_BiasedMultiHeadAtten (8-core SPMD, tensor
parallel over heads).

The torch module's transpose(0,1)+reshape "scramble" means head n of the
attention only reads rows [64n,64n+64) u [1024+64n,1024+64n+64) of q/k, and
the per-head attention factors into four 1024x1024 score blocks with
contraction 64.  Sharding 2 heads per core therefore also shards the q/k
projections 8-way (256 of 2048 rows each).

Per core c (heads 2c, 2c+1), one continuous pipeline:
  - k-proj, V^T/Ydiag seam, q-proj (bf16, PE-bound, DMA prefetched)
  - 32 attention "units" (key-block x query-chunk): scrambled scores on PE,
    exp on ACT (the only exp engine -- it is the critical resource, so
    everything else stays off ACT), AV with a ones-column in V^T producing
    softmax denominators for free
  - the gated-residual branch's matmuls ride inside the first 12 units'
    PE slack (AV is deferred there so the residual chains can hold 4 PSUM
    banks); its transposes go through the DMA xbar, its sigmoid is computed
    as 0.5*tanh(x/2)+0.5 so ACT only ever loads one table set
  - out-proj partials + output DMA ride inside the last 8 units
Host sums the 8 partial outputs with per-core row un-permutation.
"""

import numpy as np
import ml_dtypes

import concourse.bacc as bacc
import concourse.mybir as mybir
import concourse.tile as tile
from concourse import bass_utils

N_CORES = 8
L, H, E, E2, HD = 2048, 1024, 4096, 2048, 64
F32 = mybir.dt.float32
F16 = mybir.dt.float16
BF16 = mybir.dt.bfloat16
AF = mybir.ActivationFunctionType
ALU = mybir.AluOpType

_NC_CACHE = {}


def _perm16(c):
    """Block permutation: device l-tile j holds global l-tile perm[j];
    perm[0] = c and perm[1] = 8 + c so the residual rows sit at tiles 0,1."""
    perm = list(range(16))

    def place(pos, val):
        i = perm.index(val)
        perm[pos], perm[i] = perm[i], perm[pos]

    place(0, c)
    place(1, 8 + c)
    return perm


def _emit(nc, tc, d, out):
    from contextlib import ExitStack

    with ExitStack() as ctx:
        pers = ctx.enter_context(tc.tile_pool(name="pers", bufs=1))

        # ---------------- persistent SBUF tiles ----------------
        VT = [[pers.tile([128, 130], BF16, tag=f"VT{b}_{j}", name=f"VT{b}_{j}")
               for j in range(8)] for b in range(2)]
        ocolsT = pers.tile([128, 1024, 2], BF16, tag="ocolsT", name="ocolsT")
        Ydiag = [[pers.tile([128, 1024], BF16, tag=f"Yd{h}_{b}",
                            name=f"Yd{h}_{b}") for b in range(2)]
                 for h in range(2)]
        Xdup = [[pers.tile([128, 1024], BF16, tag=f"Xd{h}_{a}",
                           name=f"Xd{h}_{a}") for a in range(2)]
                for h in range(2)]
        onesrow = pers.tile([1, 64], F32, tag="onesrow", name="onesrow")
        nc.scalar.dma_start(onesrow[:], d["onesrow"][:])
        wo_sb = pers.tile([128, H], BF16, tag="wo", name="wo")
        nc.scalar.dma_start(wo_sb[:], d["WoT"][:])
        bias = {}
        for bn in ("bqb", "bkb", "blinb", "bres2b", "bob"):
            bias[bn] = pers.tile([128, H], BF16, tag=bn, name=bn)
            nc.scalar.dma_start(bias[bn][:], d[bn][:])
        resg = [pers.tile([128, H], F16, tag=f"resg{lb}", name=f"resg{lb}")
                for lb in range(2)]
        res1_sb = [pers.tile([128, H], BF16, tag=f"r1s{lb}",
                             name=f"r1s{lb}") for lb in range(2)]
        res1T = [pers.tile([128, 256], BF16, tag=f"r1_{hb}",
                           name=f"r1_{hb}") for hb in range(8)]

        # ones-columns of V^T, written once
        for b0 in range(2):
            for j in range(8):
                nc.vector.memset(VT[b0][j][:, 64:65], 1.0)
                nc.vector.memset(VT[b0][j][:, 129:130], 1.0)
        # pin the ACT table set (exp_and_others covers exp/tanh/identity)
        wz = pers.tile([128, 8], F32, tag="wz", name="wz")
        nc.vector.memset(wz[:], 0.0)
        we = pers.tile([128, 8], F32, tag="we", name="we")
        nc.scalar.activation(we[:], wz[:], AF.Exp)

        # residual weights (outlive the A-phase pools below)
        pRW = ctx.enter_context(tc.tile_pool(name="pRW", bufs=1))
        ab3 = pRW.tile([128, 16, 256], BF16, tag="ab3", name="ab3")
        wlin_sb = pRW.tile([128, 16, H], BF16, tag="wlin", name="wlin")
        wres_sb = pRW.tile([128, 8, H], BF16, tag="wres", name="wres")

        # A-phase pools (explicitly released before the attention loop)
        pN_cm = tc.tile_pool(name="pN", bufs=1)
        pN = pN_cm.__enter__()
        pW_cm = tc.tile_pool(name="pW", bufs=6)
        pW = pW_cm.__enter__()
        pAY_cm = tc.tile_pool(name="pAY", bufs=1)
        pAY = pAY_cm.__enter__()

        nodeT_sb = []
        for g in range(8):
            t = pN.tile([128, 4, 256], BF16, tag=f"node{g}", name=f"node{g}")
            nodeT_sb.append(t)
        nc.sync.dma_start(nodeT_sb[0][:], d["nodeT4"][0])

        def node_lhs(e, lb):
            return nodeT_sb[e // 4][:, e % 4, 128 * lb:128 * (lb + 1)]

        Y = [pAY.tile([128, H], BF16, tag=f"Y{b}", name=f"Y{b}")
             for b in range(2)]
        ident = pAY.tile([128, 128], BF16, tag="ident", name="ident")
        nc.scalar.dma_start(ident[:], d["ident"][:])

        # right-side PSUM stack: lets psA pop mid-attention-loop while the
        # left-side psS (allocated later) stays live
        psA_cm = tc.tile_pool(name="psA", bufs=1, space="PSUM", side="right")
        psA = psA_cm.__enter__()

        # ================= A: k-pass, VT seam, q-pass ====================
        kps = [[psA.tile([128, 512], F32, tag=f"pj{lb}{ch}",
                         name=f"k{lb}{ch}") for ch in range(2)]
               for lb in range(2)]
        wq_tiles = []
        for grp in range(16):
            wt = pW.tile([128, 2, H], BF16, tag="w", name="wk")
            nc.sync.dma_start(wt[:], d["WkT16"][grp])
            if grp < 7:
                nc.sync.dma_start(nodeT_sb[grp + 1][:], d["nodeT4"][grp + 1])
            if grp >= 8 and grp % 2 == 0:
                # prefetch the first wq groups so the q-pass starts dry
                wq = pW.tile([128, 2, H], BF16, tag="wq", name="wq")
                nc.sync.dma_start(wq[:], d["WqT16"][(grp - 8) // 2])
                wq_tiles.append(wq)
            for e2 in range(2):
                e = 2 * grp + e2
                st, sp = (e == 0), (e == 31)
                for lb in range(2):
                    lhs = node_lhs(e, lb)
                    for ch in range(2):
                        nc.tensor.matmul(
                            kps[lb][ch][:], lhs,
                            wt[:, e2, 512 * ch:512 * (ch + 1)],
                            start=st, stop=sp)
        for lb in range(2):
            for ch in range(2):
                sl = slice(512 * ch, 512 * (ch + 1))
                nc.vector.tensor_add(Y[lb][:, sl], kps[lb][ch][:],
                                     bias["bkb"][:, sl])

        # V^T tiles + Ydiag builds fill the k->q seam
        with tc.tile_pool(name="psT", bufs=4, space="PSUM") as psT:
            for b0 in range(2):
                for j in range(8):
                    pt = psT.tile([128, 128], BF16, tag="tp", name="tp")
                    nc.tensor.transpose(pt[:], Y[b0][:, 128 * j:128 * (j + 1)],
                                        ident[:])
                    vt = VT[b0][j]
                    nc.vector.tensor_copy(vt[:, 0:64], pt[:, 0:64])
                    nc.vector.tensor_copy(vt[:, 65:129], pt[:, 64:128])
            for h in range(2):
                hp = slice(64 * h, 64 * (h + 1))
                for b0 in range(2):
                    yd = Ydiag[h][b0]
                    nc.vector.memzero(yd[:])
                    src = Y[b0][hp].rearrange("p (j two c) -> p j two c",
                                              two=2, c=64)
                    dst = yd[:].rearrange("p (j two c) -> p j two c",
                                          two=2, c=64)
                    nc.gpsimd.dma_start(dst[0:64, :, 0, :], src[:, :, 0, :])
                    nc.gpsimd.dma_start(dst[64:128, :, 1, :], src[:, :, 1, :])

        qps = [[psA.tile([128, 512], F32, tag=f"pj{lb}{ch}",
                         name=f"q{lb}{ch}") for ch in range(2)]
               for lb in range(2)]
        for grp in range(16):
            if grp < len(wq_tiles):
                wt = wq_tiles[grp]
            else:
                wt = pW.tile([128, 2, H], BF16, tag="wq", name="wq")
                nc.sync.dma_start(wt[:], d["WqT16"][grp])
            for e2 in range(2):
                e = 2 * grp + e2
                st, sp = (e == 0), (e == 31)
                for lb in range(2):
                    lhs = node_lhs(e, lb)
                    for ch in range(2):
                        nc.tensor.matmul(
                            qps[lb][ch][:], lhs,
                            wt[:, e2, 512 * ch:512 * (ch + 1)],
                            start=st, stop=sp)
        for a0 in range(2):
            for ch in range(2):
                sl = slice(512 * ch, 512 * (ch + 1))
                nc.vector.tensor_add(Xdup[0][a0][0:64, sl],
                                     qps[a0][ch][0:64, :],
                                     bias["bqb"][0:64, sl])
                nc.vector.tensor_add(Xdup[1][a0][64:128, sl],
                                     qps[a0][ch][64:128, :],
                                     bias["bqb"][64:128, sl])
        for a0 in range(2):
            nc.gpsimd.dma_start(Xdup[0][a0][64:128, :], Xdup[0][a0][0:64, :])
            nc.gpsimd.dma_start(Xdup[1][a0][0:64, :], Xdup[1][a0][64:128, :])

        # residual weights queued on sync behind the projections, chunked so
        # the first rp1 steps don't wait for the whole tensor
        for c4 in range(4):
            nc.sync.dma_start(ab3[:, 4 * c4:4 * (c4 + 1), :],
                              d["abT3"][:, 4 * c4:4 * (c4 + 1), :])
            nc.sync.dma_start(wlin_sb[:, 4 * c4:4 * (c4 + 1), :],
                              d["WlinT3"][:, 4 * c4:4 * (c4 + 1), :])
        for c4 in range(2):
            nc.sync.dma_start(wres_sb[:, 4 * c4:4 * (c4 + 1), :],
                              d["WresT3"][:, 4 * c4:4 * (c4 + 1), :])

        pAY_cm.__exit__(None, None, None)
        pW_cm.__exit__(None, None, None)
        pN_cm.__exit__(None, None, None)
        psA_cm.__exit__(None, None, None)
        # right-side PSUM for the residual chains that ride inside the
        # attention units (4 banks, alongside psS's 4 on the left)
        psR_cm = tc.tile_pool(name="psR", bufs=2, space="PSUM", side="right")
        psR = psR_cm.__enter__()

        # ======= C: attention units with folded residual + out-proj ======
        oc_flat = ocolsT[:].rearrange("p a b -> p (a b)")
        psS = ctx.enter_context(tc.tile_pool(name="psS", bufs=2,
                                             space="PSUM"))
        pP = ctx.enter_context(tc.tile_pool(name="pP", bufs=24))
        pM = ctx.enter_context(tc.tile_pool(name="pM", bufs=2))
        pOB = ctx.enter_context(tc.tile_pool(name="pOB", bufs=3))

        psO_cm = tc.tile_pool(name="psO", bufs=1, space="PSUM")
        psO = None
        Ops = [None, None]
        pmap = {}

        def emit_unit(chq, bt):
            b0, jj = divmod(bt, 8)
            cq = slice(512 * chq, 512 * (chq + 1))
            ps = []
            for h in range(2):
                s = psS.tile([128, 1024], F32, tag="s", name=f"s{chq}_{bt}{h}")
                for a0 in range(2):
                    nc.tensor.matmul(
                        s[:, 512 * a0:512 * (a0 + 1)],
                        Ydiag[h][b0][:, 128 * jj:128 * (jj + 1)],
                        Xdup[h][a0][:, cq], start=True, stop=True)
                p = pP.tile([128, 1024], BF16, tag="p", name=f"p{chq}_{bt}{h}")
                nc.scalar.activation(p[:], s[:], AF.Exp, scale=0.125)
                ps.append(p)
            pmap[(chq, bt)] = ps

        def emit_av(chq, bt):
            b0, jj = divmod(bt, 8)
            for h in range(2):
                p = pmap[(chq, bt)][h]
                for a0 in range(2):
                    nc.tensor.matmul(
                        Ops[chq][a0][h][:],
                        VT[b0][jj][:, 65 * h:65 * (h + 1)],
                        p[:, 512 * a0:512 * (a0 + 1)],
                        start=(bt == 0), stop=(bt == 15))

        def emit_norm_rcp(chq):
            # DVE-only part; the PE-side apply comes later so the rb matmuls
            # never block the PE queue on this chain's latency
            rcpfs = []
            for h in range(2):
                rcp = pM.tile([1, 1024], F32, tag="rcp", name="rcp", bufs=1)
                for a0 in range(2):
                    nc.vector.tensor_copy(rcp[:, 512 * a0:512 * (a0 + 1)],
                                          Ops[chq][a0][h][64:65, :])
                rcpf = pM.tile([1, 1024], F32, tag="rcpf", name="rcpf", bufs=2)
                nc.vector.reciprocal_approx_fast(rcpf[:], rcp[:])
                rcpfs.append(rcpf)
            return rcpfs

        def emit_norm_apply(chq, rcpfs):
            cq = slice(512 * chq, 512 * (chq + 1))
            for h in range(2):
                rcpf = rcpfs[h]
                rb = psS.tile([64, 1024], F32, tag="s", name="rb")
                for a0 in range(2):
                    nc.tensor.matmul(rb[:, 512 * a0:512 * (a0 + 1)],
                                     onesrow[:],
                                     rcpf[:, 512 * a0:512 * (a0 + 1)],
                                     start=True, stop=True)
                rbs = pM.tile([64, 1024], F32, tag="rbs", name="rbs", bufs=1)
                nc.vector.tensor_copy(rbs[:], rb[:])
                for a0 in range(2):
                    nc.vector.tensor_mul(
                        ocolsT[64 * h:64 * (h + 1), cq, a0],
                        Ops[chq][a0][h][0:64, :],
                        rbs[0:64, 512 * a0:512 * (a0 + 1)])

        def emit_out(j, act_copy=False, otags=None):
            if otags is None:
                op = psS.tile([128, 1024], F32, tag="s", name=f"op{j}")
                halves = [op[:, 0:512], op[:, 512:1024]]
            else:
                halves = [psO.tile([128, 512], F32, tag=t, name=f"op{j}{t}")[:]
                          for t in otags]
            for ch in range(2):
                nc.tensor.matmul(halves[ch],
                                 oc_flat[:, 128 * j:128 * (j + 1)],
                                 wo_sb[:, 512 * ch:512 * (ch + 1)],
                                 start=True, stop=True)
            ob = pOB.tile([128, H], F16, tag="ob", name=f"ob{j}")
            for ch in range(2):
                sl = slice(512 * ch, 512 * (ch + 1))
                if j < 2:
                    nc.vector.tensor_add(ob[:, sl], halves[ch], resg[j][:, sl])
                elif act_copy:
                    nc.scalar.activation(ob[:, sl], halves[ch], AF.Identity)
                else:
                    nc.vector.tensor_copy(ob[:, sl], halves[ch])
            nc.sync.dma_start(out[128 * j:128 * (j + 1), :], ob[:])

        def rp1t_chain(hb):
            # res1^T[hb] = (ab @ Wlin^T)^T chunk, produced transposed so rp2
            # never waits on a transpose
            rpt = psR.tile([128, 256], F32, tag="r1", name=f"rpt{hb}")
            for t in range(16):
                nc.tensor.matmul(rpt[:],
                                 wlin_sb[:, t, 128 * hb:128 * (hb + 1)],
                                 ab3[:, t, :], start=(t == 0), stop=(t == 15))
            nc.vector.tensor_copy(res1T[hb][:], rpt[:])

        tt = None
        RP1_HB = {0: [0, 1], 1: [2], 2: [3], 3: [4, 5], 4: [6], 5: [7]}
        OP0_J = [2, 3, 4, 5, 6, 7, 0, 1]
        for u in range(32):
            chq, bt = divmod(u, 16)
            emit_unit(chq, bt)
            if u <= 5:
                for hb in RP1_HB[u]:
                    rp1t_chain(hb)
            if 7 <= u <= 14:
                # lazily rebuild row-major res1 (only the resg gate needs it)
                hb = u - 7
                for lb in range(2):
                    nc.sync.dma_start_transpose(
                        res1_sb[lb][:, 128 * hb:128 * (hb + 1)],
                        res1T[hb][:, 128 * lb:128 * (lb + 1)])
            if 6 <= u <= 9:
                ch = (u - 6) // 2
                if u % 2 == 0:
                    rp2 = [psR.tile([128, 512], F32, tag="r2",
                                    name=f"rp2{lb}{ch}") for lb in range(2)]
                    if ch == 0:
                        tt = [pM.tile([128, H], F32, tag=f"tt{lb}",
                                      name=f"tt{lb}", bufs=1)
                              for lb in range(2)]
                for hb in range(4 * (u % 2), 4 * (u % 2) + 4):
                    for lb in range(2):
                        nc.tensor.matmul(
                            rp2[lb][:],
                            res1T[hb][:, 128 * lb:128 * (lb + 1)],
                            wres_sb[:, hb, 512 * ch:512 * (ch + 1)],
                            start=(hb == 0), stop=(hb == 7))
                if u % 2 == 1:
                    sl = slice(512 * ch, 512 * (ch + 1))
                    for lb in range(2):
                        nc.vector.tensor_add(tt[lb][:, sl], rp2[lb][:],
                                             bias["bres2b"][:, sl])
            elif u == 10:
                gtiles = []
                for lb in range(2):
                    g = pM.tile([128, H], F32, tag=f"g{lb}", name=f"g{lb}",
                                bufs=1)
                    # sigmoid(x) = 0.5*tanh(x/2) + 0.5 (same ACT table set)
                    nc.scalar.activation(g[:], tt[lb][:], AF.Tanh, scale=0.5)
                    nc.vector.tensor_scalar(g[:], g[:], 0.5, 0.5,
                                            ALU.mult, ALU.add)
                    gtiles.append(g)
            elif u == 11:
                psR_cm.__exit__(None, None, None)
                psO = psO_cm.__enter__()
                Ops[0] = [[psO.tile([65, 512], F32, tag=f"o{a0}{h}",
                                    name=f"O0_{a0}{h}") for h in range(2)]
                          for a0 in range(2)]
            elif 12 <= u <= 19:
                for b in (2 * (u - 12), 2 * (u - 12) + 1):
                    emit_av(0, b)
                if u == 15:
                    for lb in range(2):
                        nc.vector.tensor_add(resg[lb][:], res1_sb[lb][:],
                                             bias["blinb"][:])
                        nc.vector.tensor_mul(resg[lb][:], resg[lb][:],
                                             gtiles[lb][:])
                        nc.vector.tensor_add(resg[lb][:], resg[lb][:],
                                             bias["bob"][:])
                if u == 19:
                    rcpf0 = emit_norm_rcp(0)
                    Ops[1] = [[psO.tile([65, 512], F32, tag=f"o{a0}{h}",
                                        name=f"O1_{a0}{h}") for h in range(2)]
                              for a0 in range(2)]
            elif 20 <= u <= 23:
                for b in (2 * (u - 20), 2 * (u - 20) + 1):
                    emit_av(1, b)
                if u == 21:
                    emit_norm_apply(0, rcpf0)
            elif 24 <= u <= 31:
                emit_av(1, u - 16)
                emit_out(OP0_J[u - 24])
        rcpf1 = emit_norm_rcp(1)
        emit_norm_apply(1, rcpf1)
        for j in range(8, 16):
            otags = None
            if j % 2 == 1:
                otags = ("o00", "o01") if j % 4 == 1 else ("o10", "o11")
            emit_out(j, act_copy=(j % 2 == 0), otags=otags)
        psO_cm.__exit__(None, None, None)


def _build_nc():
    nc = bacc.Bacc("TRN2", target_bir_lowering=False, debug=False,
                   num_devices=N_CORES)
    d = {}

    def din(name, shape, dt=BF16):
        d[name] = nc.dram_tensor(name, shape, dt, kind="ExternalInput").ap()

    din("nodeT4", (8, 128, 4, 256))
    din("WqT16", (16, 128, 2, H))
    din("WkT16", (16, 128, 2, H))
    din("abT3", (128, 16, 256))
    din("WlinT3", (128, 16, H))
    din("WresT3", (128, 8, H))
    din("WoT", (128, H))
    din("ident", (128, 128))
    din("onesrow", (1, 64), F32)
    for bn in ("bqb", "bkb", "blinb", "bres2b", "bob"):
        din(bn, (128, H))
    out = nc.dram_tensor("out", (L, H), F16, kind="ExternalOutput").ap()
    with tile.TileContext(nc) as tc:
        _emit(nc, tc, d, out)
    nc.compile()
    return nc


def get_nc():
    if "nc" not in _NC_CACHE:
        _NC_CACHE["nc"] = _build_nc()
    return _NC_CACHE["nc"]


def build_in_maps(inputs):
    f32 = np.float32
    bf16 = ml_dtypes.bfloat16
    ne = np.asarray(inputs["node_embedding"], f32)
    ab = np.asarray(inputs["atten_bias"], f32)
    Wq = np.asarray(inputs["Wq"], f32)
    Wk = np.asarray(inputs["Wk"], f32)
    Wlin = np.asarray(inputs["Wlin"], f32)
    Wres = np.asarray(inputs["Wres"], f32)
    Wo = np.asarray(inputs["Wo"], f32)
    bq = np.asarray(inputs["bq"], f32)
    bk = np.asarray(inputs["bk"], f32)
    blin = np.asarray(inputs["blin"], f32)
    bres = np.asarray(inputs["bres"], f32)
    bo = np.asarray(inputs["bo"], f32)

    WkT16 = np.ascontiguousarray(
        Wk.T.reshape(16, 2, 128, H).transpose(0, 2, 1, 3)).astype(bf16)
    WlinT3 = np.ascontiguousarray(
        Wlin.T.reshape(16, 128, H).transpose(1, 0, 2)).astype(bf16)
    WresT3 = np.ascontiguousarray(
        Wres.T.reshape(8, 128, H).transpose(1, 0, 2)).astype(bf16)
    ident = np.eye(128, dtype=f32).astype(bf16)
    bres2 = (Wres @ blin + bres).astype(f32)

    def rep(x):
        return np.ascontiguousarray(
            np.broadcast_to(x.reshape(1, H), (128, H))).astype(bf16)

    in_maps = []
    for c in range(N_CORES):
        rows = np.r_[128 * c:128 * (c + 1),
                     1024 + 128 * c:1024 + 128 * (c + 1)]
        colperm = np.concatenate([np.arange(64) + 64 * p for p in _perm16(c)])
        in_maps.append({
            "nodeT4": np.ascontiguousarray(
                ne[rows].T.reshape(8, 4, 128, 256).transpose(
                    0, 2, 1, 3)).astype(bf16),
            "WqT16": np.ascontiguousarray(
                Wq.T[:, colperm].reshape(16, 2, 128, H).transpose(
                    0, 2, 1, 3)).astype(bf16),
            "WkT16": WkT16,
            "abT3": np.ascontiguousarray(
                ab[rows].T.reshape(16, 128, 256).transpose(
                    1, 0, 2)).astype(bf16),
            "WlinT3": WlinT3,
            "WresT3": WresT3,
            "WoT": np.ascontiguousarray(
                Wo[:, 128 * c:128 * (c + 1)].T).astype(bf16),
            "ident": ident,
            "onesrow": np.ones((1, 64), f32),
            "bqb": rep(bq[colperm]),
            "bkb": rep(bk),
            "blinb": rep(blin),
            "bres2b": rep(bres2),
            "bob": rep(bo),
        })
    return in_maps


def combine_outputs(results):
    full = np.zeros((L, H), np.float32)
    for c in range(N_CORES):
        o = np.asarray(results[c]["out"], np.float32)
        perm = _perm16(c)
        for j in range(16):
            full[128 * perm[j]:128 * (perm[j] + 1)] += o[128 * j:128 * (j + 1)]
    return full


def kernel(**inputs):
    nc = get_nc()
    in_maps = build_in_maps(inputs)
    res = bass_utils.run_bass_kernel_spmd(nc, in_maps,
                                          core_ids=list(range(N_CORES)))
    return combine_outputs(res.results)
